# revision 4
# baseline (speedup 1.0000x reference)
"""Mixed-precision quantized linear on 8 trn2 cores — v8.

v4 numerics (bf16 x for q8/p2 groups, fp8e4 DoubleRow for w16/p4 with
e4m3 x; weights shipped unpacked in fp8/int8) with the v6 scheduling
lessons applied:
- every DVE op runs on a FLAT 2D contiguous slice (3D access patterns
  drop DVE to 1x and slower),
- fine-grained DMA stream on the sync ring in PE-consumption order,
- output DMAs on the scalar ring so they don't block input FIFO,
- dummy warmup matmuls hold the PE HAM clock at 2.4GHz until data lands,
- per-group K=1 bias matmul (ones row x bias row), epilogue scale on DVE.

Channel order per core: [q8 384 | p2 256 | w16 128 | p4 512].
"""

import numpy as np
import ml_dtypes

import concourse.bass as bass
import concourse.bacc as bacc
import concourse.mybir as mybir
import concourse.tile as tile
from concourse.bass_utils import run_bass_kernel_spmd

IN = 4096
OUT = 11008
N16, N8, N4, N2 = 1024, 3072, 4096, 2048
M = 256
NCORES = 8
C16, C8, C4, C2 = N16 // 8, N8 // 8, N4 // 8, N2 // 8  # 128, 384, 512, 256
NCH = C16 + C8 + C4 + C2  # 1280
KT = IN // 128  # 32

BF16 = mybir.dt.bfloat16
F32 = mybir.dt.float32
I8 = mybir.dt.int8
F8E4 = mybir.dt.float8e4

Alu = mybir.AluOpType
DR = mybir.MatmulPerfMode.DoubleRow

bf16 = ml_dtypes.bfloat16
e4m3 = ml_dtypes.float8_e4m3

SEG_A1, SEG_A2, SEG_B1, SEG_B2 = 0, C8, C8 + C2, C8 + C2 + C16

WARMUP_MMS = 48


def _build_nc():
    nc = bacc.Bacc()
    xt_d = nc.declare_dram_parameter("xt", [128, 2 * KT * 128], BF16, isOutput=False)
    xf_d = nc.declare_dram_parameter("xf", [128, 2 * KT * 128], F8E4, isOutput=False)
    q8_d = nc.declare_dram_parameter("q8w", [128, KT * C8], I8, isOutput=False)
    w16_d = nc.declare_dram_parameter("w16f", [128, KT * C16], F8E4, isOutput=False)
    p4_d = nc.declare_dram_parameter("p4f", [128, KT * C4], F8E4, isOutput=False)
    p2_d = nc.declare_dram_parameter("p2f", [128, KT * C2], F8E4, isOutput=False)
    sbr_d = nc.declare_dram_parameter("sbr", [128, NCH], BF16, isOutput=False)
    br_d = nc.declare_dram_parameter("brow", [1, NCH], BF16, isOutput=False)
    out_d = nc.declare_dram_parameter("out", [M, NCH], BF16, isOutput=True)

    with tile.TileContext(nc) as tc:
        with (
            tc.tile_pool(name="big", bufs=1) as pool,
            tc.tile_pool(name="psum", bufs=1, space="PSUM") as ppool,
        ):
            xs = pool.tile([128, 2 * KT * 128], BF16)
            xf = pool.tile([128, 2 * KT * 128], F8E4)
            q8i = pool.tile([128, KT * C8], I8)
            q8s = pool.tile([128, KT * C8], BF16)
            w16s = pool.tile([128, KT * C16], F8E4)
            p4s = pool.tile([128, KT * C4], F8E4)
            p2s = pool.tile([128, KT * C2], F8E4)
            sbcs = pool.tile([128, NCH], BF16)
            brs = pool.tile([1, NCH], BF16)
            brs2 = pool.tile([1, NCH], BF16)
            ones_b = pool.tile([1, 128], BF16)
            outs = pool.tile([128, 2 * NCH], BF16)
            wma = pool.tile([128, 128], BF16)
            wmb = pool.tile([128, 256], BF16)

            XB = KT * 128  # 4096, one block of x columns

            # ---- DMA stream (sync ring), fat transfers in consumption order
            S = nc.sync
            S.dma_start(out=brs[:], in_=br_d[:])
            S.dma_start(out=sbcs[:], in_=sbr_d[:])
            S.dma_start(out=xf[:], in_=xf_d[:])
            S.dma_start(out=p4s[:, : KT * C4 // 2], in_=p4_d[:, : KT * C4 // 2])
            S.dma_start(out=w16s[:], in_=w16_d[:])
            S.dma_start(out=p4s[:, KT * C4 // 2 :], in_=p4_d[:, KT * C4 // 2 :])
            qq = KT * C8 // 4
            S.dma_start(out=q8i[:, :qq], in_=q8_d[:, :qq])
            S.dma_start(out=q8i[:, qq : 2 * qq], in_=q8_d[:, qq : 2 * qq])
            S.dma_start(out=xs[:, :XB], in_=xt_d[:, :XB])
            S.dma_start(out=p2s[:], in_=p2_d[:])
            S.dma_start(out=xs[:, XB:], in_=xt_d[:, XB:])
            S.dma_start(out=q8i[:, 2 * qq : 3 * qq], in_=q8_d[:, 2 * qq : 3 * qq])
            S.dma_start(out=q8i[:, 3 * qq :], in_=q8_d[:, 3 * qq :])

            # ---- DVE: q8 cast quarters (flat), tiny row copy, warmup memsets
            nc.vector.memset(wma[:], 0.0)
            nc.vector.memset(wmb[:], 0.0)
            nc.vector.tensor_copy(brs2[:], brs[:])
            nc.vector.memset(ones_b[:], 1.0)
            for c in range(4):
                nc.vector.tensor_copy(q8s[:, c * qq : (c + 1) * qq],
                                      q8i[:, c * qq : (c + 1) * qq])

            ps = {}
            for b in range(2):
                ps[b, "a1"] = ppool.tile([128, C8], F32, name=f"psa1_{b}")
                ps[b, "a2"] = ppool.tile([128, C2], F32, name=f"psa2_{b}")
                ps[b, "b1"] = ppool.tile([128, C16], F32, name=f"psb1_{b}")
                ps[b, "b2"] = ppool.tile([128, C4], F32, name=f"psb2_{b}")

            # ---- PE program
            for i in range(WARMUP_MMS):
                nc.tensor.matmul(
                    ps[1, "b2"][:, :256], wma[:], wmb[:],
                    start=True, stop=(i == WARMUP_MMS - 1), skip_group_check=True,
                )

            def two(ap):
                return ap.rearrange("p (two n) -> p two n", two=2)

            def dr_half(b, h):
                for t in range(h * 8, h * 8 + 8):
                    lhs = two(xf[:, b * XB + 2 * t * 128 : b * XB + 2 * t * 128 + 256])
                    nc.tensor.matmul(
                        ps[b, "b2"][:, :],
                        lhs, two(p4s[:, 2 * t * C4 : (2 * t + 2) * C4]),
                        start=(t == 0), stop=False, perf_mode=DR,
                        skip_group_check=True,
                    )
                    nc.tensor.matmul(
                        ps[b, "b1"][:, :],
                        lhs, two(w16s[:, 2 * t * C16 : (2 * t + 2) * C16]),
                        start=(t == 0), stop=False, perf_mode=DR,
                        skip_group_check=True,
                    )

            def a_part(b, key, w, cw, k0, k1):
                for kt in range(k0, k1):
                    nc.tensor.matmul(
                        ps[b, key][:, :],
                        xs[:, (b * KT + kt) * 128 : (b * KT + kt) * 128 + 128],
                        w[:, kt * cw : (kt + 1) * cw],
                        start=(kt == 0), stop=False, skip_group_check=True,
                    )

            seg = {"a1": SEG_A1, "a2": SEG_A2, "b1": SEG_B1, "b2": SEG_B2}

            def finish(b, keys):
                for key in keys:
                    c0 = seg[key]
                    cw = ps[b, key].shape[-1]
                    nc.tensor.matmul(
                        ps[b, key][:, :], ones_b[:1, :], brs2[:1, c0 : c0 + cw],
                        start=False, stop=True, skip_group_check=True,
                    )
                    nc.vector.scalar_tensor_tensor(
                        outs[:, b * NCH + c0 : b * NCH + c0 + cw],
                        ps[b, key][:, :], 1.0, sbcs[:, c0 : c0 + cw],
                        op0=Alu.mult, op1=Alu.mult,
                    )

            out_v = out_d[:].rearrange("(b p) n -> p b n", p=128)

            dr_half(0, 0)
            dr_half(0, 1)
            dr_half(1, 0)
            dr_half(1, 1)
            finish(0, ["b2", "b1"])
            finish(1, ["b2", "b1"])
            # keep-warm burst: bridge the q8-cast wait without a HAM MID window
            for i in range(10):
                nc.tensor.matmul(
                    ps[1, "a2"][:, :], wma[:], wmb[:],
                    start=True, stop=(i == 9), skip_group_check=True,
                )
            a_part(0, "a1", q8s, C8, 0, 8)
            a_part(0, "a1", q8s, C8, 8, 16)
            a_part(0, "a2", p2s, C2, 0, 32)
            a_part(1, "a1", q8s, C8, 0, 8)
            a_part(1, "a1", q8s, C8, 8, 16)
            a_part(0, "a1", q8s, C8, 16, 24)
            a_part(1, "a1", q8s, C8, 16, 24)
            a_part(0, "a1", q8s, C8, 24, 32)
            a_part(1, "a1", q8s, C8, 24, 32)
            finish(0, ["a1", "a2"])
            nc.scalar.dma_start(out=out_v[:, 0, :], in_=outs[:, :NCH])
            a_part(1, "a2", p2s, C2, 0, 32)
            finish(1, ["a2"])
            nc.scalar.dma_start(
                out=out_v[:, 1, SEG_A2:], in_=outs[:, NCH + SEG_A2 :]
            )
            finish(1, ["a1"])
            nc.scalar.dma_start(
                out=out_v[:, 1, :SEG_A2], in_=outs[:, NCH : NCH + SEG_A2]
            )
    nc.finalize()
    return nc


def _ktile(a):
    """[K, F] -> [128, (K/128)*F] matching flat SBUF [128, kt*F]."""
    k, f = a.shape
    t = k // 128
    return np.ascontiguousarray(
        a.reshape(t, 128, f).transpose(1, 0, 2).reshape(128, t * f)
    )


def _unpack4(p):
    u = p.astype(np.uint8)
    lo = (u & 15).astype(np.int32)
    hi = ((u >> 4) & 15).astype(np.int32)
    full = np.stack([lo, hi], -1).reshape(p.shape[0], -1)
    return np.where(full > 7, full - 16, full).astype(np.float32)


_CACHE = {}


def stage_inputs(**inputs):
    x = np.asarray(inputs["x"], dtype=np.float32)
    w16 = np.asarray(inputs["w16"], dtype=np.float32)
    b16 = np.asarray(inputs["b16"], dtype=np.float32)
    q8 = np.asarray(inputs["q8"])
    s8 = np.asarray(inputs["s8"], dtype=np.float32)
    b8 = np.asarray(inputs["b8"], dtype=np.float32)
    s4 = np.asarray(inputs["s4"], dtype=np.float32)
    b4 = np.asarray(inputs["b4"], dtype=np.float32)
    s2 = np.asarray(inputs["s2"], dtype=np.float32)
    b2 = np.asarray(inputs["b2"], dtype=np.float32)

    xT = np.ascontiguousarray(x.T)  # [4096, 256]
    t = xT.reshape(KT, 128, 2, 128).transpose(1, 2, 0, 3)  # [p, blk, kt, tok]
    xt = np.ascontiguousarray(t.reshape(128, 2 * KT * 128)).astype(bf16)
    xf = xt.astype(np.float32).astype(e4m3)

    w4i = _unpack4(np.asarray(inputs["p4"]))
    w2i = _unpack4(np.asarray(inputs["p2"]))
    rs16 = 128.0 / np.maximum(np.abs(w16).max(axis=1), 1e-30)

    in_maps = []
    cat_idxs = []
    for k in range(NCORES):
        sl16 = slice(k * C16, (k + 1) * C16)
        sl8 = slice(k * C8, (k + 1) * C8)
        sl4 = slice(k * C4, (k + 1) * C4)
        sl2 = slice(k * C2, (k + 1) * C2)

        q8w = _ktile(np.ascontiguousarray(q8[sl8].astype(np.int8).T)).astype(np.int8)
        w16f = _ktile(
            np.ascontiguousarray((w16[sl16] * rs16[sl16][:, None]).T)
        ).astype(e4m3)
        p4f = _ktile(np.ascontiguousarray(w4i[sl4].T)).astype(e4m3)
        p2f = _ktile(np.ascontiguousarray(w2i[sl2].T)).astype(e4m3)

        srow = np.concatenate(
            [s8[sl8, 0], s2[sl2, 0], 1.0 / rs16[sl16], s4[sl4, 0]]
        )
        sbr = np.ascontiguousarray(
            np.broadcast_to(srow[None, :].astype(bf16), (128, NCH))
        )
        brow = (
            np.concatenate(
                [
                    b8[sl8] / s8[sl8, 0],
                    b2[sl2] / s2[sl2, 0],
                    b16[sl16] * rs16[sl16],
                    b4[sl4] / s4[sl4, 0],
                ]
            )
            .reshape(1, NCH)
            .astype(bf16)
        )

        in_maps.append(
            {"xt": xt, "xf": xf, "q8w": q8w, "w16f": w16f, "p4f": p4f,
             "p2f": p2f, "sbr": sbr, "brow": brow}
        )
        cat_idxs.append(
            np.concatenate(
                [
                    np.asarray(inputs["idx8"])[sl8],
                    np.asarray(inputs["idx2"])[sl2],
                    np.asarray(inputs["idx16"])[sl16],
                    np.asarray(inputs["idx4"])[sl4],
                ]
            )
        )
    return in_maps, cat_idxs


def kernel(**inputs):
    in_maps, cat_idxs = stage_inputs(**inputs)
    if "nc" not in _CACHE:
        _CACHE["nc"] = _build_nc()
    res = run_bass_kernel_spmd(_CACHE["nc"], in_maps, core_ids=list(range(NCORES)))
    _CACHE["last_res"] = res

    out = np.zeros((M, OUT), dtype=np.float32)
    for k in range(NCORES):
        out[:, cat_idxs[k]] = res.results[k]["out"].astype(np.float32)
    return out
